# revision 1
# baseline (speedup 1.0000x reference)
"""FrequencyStream Trainium2 kernel (8 NeuronCores, SPMD).

Pipeline per reference nn.Module:
  x [32,3,224,224] -> 2D DCT-II per channel -> conv3x3(3->64)+relu+maxpool2
  -> conv3x3(64->128)+relu+maxpool2 -> conv3x3(128->256)+relu+maxpool2
  -> flatten -> fc [512, 200704] -> [32, 512]

Distribution:
  - DCT+convs data-parallel over batch: 4 images per core.
  - FC tensor-parallel over the contraction: an AllToAll exchanges h chunks
    so core j holds h[:, 25088*j:25088*(j+1)] for all 32 images and
    multiplies against its fcw column slice (host sums the 8 partials).

Matmuls run as float32r (full PE rate, ~1e-4 rel err). The FC runs in bf16
(it is weight-DMA bound; bf16 halves the 411MB weight stream).
"""

import numpy as np

_CACHE = {}

N = 224          # spatial size
P = 112          # DCT partition tile (224 = 2*112)
NIMG = 4         # images per core
NCORES = 8
KCH = 25088      # fc contraction chunk per core (= 32 channels * 784)
KT = KCH // 128  # 196 fc k-tiles per core


def _dct_matrix(n):
    k = np.arange(n)[:, None].astype(np.float64)
    m = np.arange(n)[None, :].astype(np.float64)
    D = np.sqrt(2.0 / n) * np.cos(np.pi * (2 * m + 1) * k / (2 * n))
    D[0, :] *= 1.0 / np.sqrt(2.0)
    return D.astype(np.float32)


def _build(sim_single=False):
    import concourse.bass as bass
    import concourse.tile as tile
    from concourse import bacc, mybir
    from concourse.masks import make_identity

    F32 = mybir.dt.float32
    BF16 = mybir.dt.bfloat16
    R = mybir.dt.float32r
    MAX = mybir.AluOpType.max
    RELU = mybir.ActivationFunctionType.Relu

    nc = bacc.Bacc("TRN2", target_bir_lowering=False, debug=False,
                   num_devices=1 if sim_single else NCORES)

    x4 = nc.dram_tensor("x4", (NIMG, 3, N, N), R, kind="ExternalInput").ap()
    dctT = nc.dram_tensor("dctT", (P, 2, 256), R, kind="ExternalInput").ap()
    w1d = nc.dram_tensor("w1d", (27, 64), R, kind="ExternalInput").ap()
    w2pd = nc.dram_tensor("w2pd", (128, 3, 128), R, kind="ExternalInput").ap()
    w2ld = nc.dram_tensor("w2ld", (64, 3, 128), R, kind="ExternalInput").ap()
    w3d = nc.dram_tensor("w3d", (128, 9, 256), R, kind="ExternalInput").ap()
    b1d = nc.dram_tensor("b1d", (64, 1), F32, kind="ExternalInput").ap()
    b2d = nc.dram_tensor("b2d", (128, 1), F32, kind="ExternalInput").ap()
    b3d = nc.dram_tensor("b3d", (128, 2), F32, kind="ExternalInput").ap()
    fcwT = nc.dram_tensor("fcwT", (KCH, 512), BF16, kind="ExternalInput").ap()
    partial = nc.dram_tensor("partial", (32, 512), F32,
                             kind="ExternalOutput").ap()

    cc_in = nc.dram_tensor("cc_in", (NCORES, NIMG, KCH), F32,
                           kind="Internal").ap()
    cc_out = nc.dram_tensor("cc_out", (NCORES, NIMG, KCH), F32,
                            kind="Internal").ap()

    taps9 = [(dy, dx) for dy in range(3) for dx in range(3)]

    with tile.TileContext(nc) as tc:
        with tc.tile_pool(name="const", bufs=1) as const, \
             tc.tile_pool(name="sbA", bufs=2) as sbA, \
             tc.tile_pool(name="act1p", bufs=1) as act1p, \
             tc.tile_pool(name="act2p", bufs=1) as act2p, \
             tc.tile_pool(name="act3p", bufs=1) as act3p, \
             tc.tile_pool(name="sbim", bufs=3) as sbim, \
             tc.tile_pool(name="dramp", bufs=2, space="DRAM") as dramp, \
             tc.tile_pool(name="psD", bufs=2, space="PSUM") as psD, \
             tc.tile_pool(name="psC1", bufs=2, space="PSUM") as psC1, \
             tc.tile_pool(name="psC23", bufs=2, space="PSUM") as psC23:

            # ---- constants ----
            DTt = const.tile([P, 2, 256], R)
            nc.sync.dma_start(DTt[:], dctT)
            w1t = const.tile([27, 64], R)
            nc.sync.dma_start(w1t[:], w1d)
            w2p = const.tile([128, 3, 128], R)
            nc.sync.dma_start(w2p[:], w2pd)
            w2l = const.tile([64, 3, 128], R)
            nc.sync.dma_start(w2l[:], w2ld)
            w3t = const.tile([128, 9, 256], R)
            nc.sync.dma_start(w3t[:], w3d)
            b1t = const.tile([64, 1], F32)
            nc.sync.dma_start(b1t[:], b1d)
            b2t = const.tile([128, 1], F32)
            nc.sync.dma_start(b2t[:], b2d)
            b3t = const.tile([128, 2], F32)
            nc.sync.dma_start(b3t[:], b3d)
            ztf = const.tile([128, 256], F32)
            nc.vector.memset(ztf[:], 0.0)
            zt = const.tile([128, 256], R)
            nc.scalar.copy(zt[:], ztf[:])

            def dct(i):
                """image i -> freq_pad DRAM tile [3, 226, 226] (zero border)."""
                Xt = sbA.tile([P, 3, 2, N], R, tag="X")
                nc.sync.dma_start(
                    Xt[:], x4[i].rearrange("c (kt p) n -> p c kt n", p=P))
                Tt = sbA.tile([P, 2, 3, 256], R, tag="T")
                for c in range(3):
                    for nt in range(2):
                        ps = psD.tile([P, 256], F32, tag="dct")
                        for kt in range(2):
                            nc.tensor.matmul(
                                ps[:],
                                lhsT=Xt[:, c, kt, nt * P:(nt + 1) * P],
                                rhs=DTt[:, kt, :],
                                start=(kt == 0), stop=(kt == 1))
                        nc.scalar.copy(Tt[:, nt, c, :], ps[:])
                fq = sbA.tile([P, 2, 3, N], R, tag="fq")
                for c in range(3):
                    for ht in range(2):
                        ps = psD.tile([P, 256], F32, tag="dct")
                        for kt in range(2):
                            nc.tensor.matmul(
                                ps[:],
                                lhsT=Tt[:, kt, c, ht * P:(ht + 1) * P],
                                rhs=DTt[:, kt, :],
                                start=(kt == 0), stop=(kt == 1))
                        nc.scalar.copy(fq[:, ht, c, :], ps[:, 0:N])
                fp = dramp.tile([3, 226, 226], R, tag="freqpad")
                nc.scalar.dma_start(fp[:, 0, :], zt[0:3, 0:226])
                nc.scalar.dma_start(fp[:, 225, :], zt[0:3, 0:226])
                nc.scalar.dma_start(fp[:, :, 0], zt[0:3, 0:226])
                nc.scalar.dma_start(fp[:, :, 225], zt[0:3, 0:226])
                for ht in range(2):
                    nc.scalar.dma_start(
                        fp[:, 1 + P * ht:1 + P * (ht + 1), 1:225]
                        .rearrange("c p x -> p c x"),
                        fq[:, ht])
                return fp

            def conv1(i, fp):
                """freq_pad -> act1 [128 = 64ch | 64ch shifted +1 row, 114, 114]."""
                act1 = act1p.tile([128, 114, 114], R, tag="act1")
                nc.gpsimd.tensor_copy(act1[0:64, 0, :], zt[0:64, 0:114])
                nc.gpsimd.tensor_copy(act1[0:64, 113, :], zt[0:64, 0:114])
                nc.gpsimd.tensor_copy(act1[:, :, 0], zt[:, 0:114])
                nc.gpsimd.tensor_copy(act1[:, :, 113], zt[:, 0:114])
                fpap = fp[:]
                for blk in range(14):          # 16 conv-out rows per block
                    Y0 = 16 * blk
                    r1 = sbim.tile([27, 16, N], R, tag="r1")
                    rv = r1[:].rearrange("(c t) yy x -> c t yy x", t=9)
                    # im2col: partition (c,dy,dx): elem (yy,x) = fp[c, Y0+dy+yy, dx+x]
                    qeng = [nc.sync, nc.scalar, nc.gpsimd]
                    for dy in range(3):
                        for dx in range(3):
                            src = bass.AP(
                                tensor=fpap.tensor,
                                offset=fpap.offset + (Y0 + dy) * 226 + dx,
                                ap=[[226 * 226, 3], [226, 16], [1, N]])
                            qeng[dy].dma_start(rv[:, dy * 3 + dx], src)
                    for g4 in range(4):        # psum groups of 4 rows
                        yl = 4 * g4
                        ps = psC1.tile([64, 2, 512], F32, tag="c1")
                        for ch in range(2):
                            nc.tensor.matmul(
                                ps[:, ch, 0:448],
                                lhsT=w1t[:],
                                rhs=r1[:, yl + 2 * ch:yl + 2 * ch + 2,
                                       :],
                                start=True, stop=True)
                        v = ps[:, :, 0:448].rearrange(
                            "p ch (yy xp two) -> p ch xp yy two", yy=2, xp=P, two=2)
                        m2 = sbim.tile([64, 2, P], F32, tag="m2")
                        nc.vector.tensor_reduce(m2[:], v, mybir.AxisListType.XY,
                                                MAX)
                        ro = 8 * blk + 2 * g4
                        nc.scalar.activation(act1[0:64, 1 + ro:3 + ro, 1:113],
                                             m2[:], RELU, bias=b1t[:, 0:1])
                        nc.vector.tensor_copy(act1[64:128, ro:ro + 2, 1:113],
                                              act1[0:64, 1 + ro:3 + ro, 1:113])
                return act1

            def conv2(i, act1):
                act2 = act2p.tile([128, 58, 58], R, tag="act2")
                nc.vector.tensor_copy(act2[:, 0, :], zt[:, 0:58])
                nc.vector.tensor_copy(act2[:, 57, :], zt[:, 0:58])
                nc.vector.tensor_copy(act2[:, :, 0], zt[:, 0:58])
                nc.vector.tensor_copy(act2[:, :, 57], zt[:, 0:58])
                for g in range(28):            # 4 out rows per chunk
                    y0 = 4 * g
                    ps = psC23.tile([128, 512], F32, tag="c23")
                    for dx in range(3):        # dy 0&1 pairs via shifted dup
                        nc.tensor.matmul(
                            ps[:, 0:448],
                            lhsT=w2p[:, dx, :],
                            rhs=act1[:, y0:y0 + 4, dx:dx + P],
                            start=(dx == 0), stop=False)
                    for dx in range(3):        # dy=2 leftover, K=64
                        nc.tensor.matmul(
                            ps[:, 0:448],
                            lhsT=w2l[:, dx, :],
                            rhs=act1[0:64, y0 + 2:y0 + 6, dx:dx + P],
                            start=False, stop=(dx == 2))
                    v = ps[:, 0:448].rearrange(
                        "p (yp yb xp two) -> p yp xp yb two", yp=2, yb=2, two=2)
                    m2 = sbim.tile([128, 2, 56], F32, tag="m22")
                    nc.vector.tensor_reduce(m2[:], v, mybir.AxisListType.XY,
                                            MAX)
                    ro = 2 * g
                    nc.scalar.activation(act2[:, 1 + ro:3 + ro, 1:57],
                                         m2[:], RELU, bias=b2t[:, 0:1])
                return act2

            def conv3(i, act2):
                act3 = act3p.tile([128, 2, 28, 28], R, tag="act3")
                for mt in range(2):
                    for g in range(7):         # 8 out rows per chunk
                        y0 = 8 * g
                        ps = psC23.tile([128, 512], F32, tag="c23")
                        for t, (dy, dx) in enumerate(taps9):
                            nc.tensor.matmul(
                                ps[:, 0:448],
                                lhsT=w3t[:, t, mt * 128:(mt + 1) * 128],
                                rhs=act2[:, y0 + dy:y0 + dy + 8,
                                         dx:dx + 56],
                                start=(t == 0), stop=(t == 8))
                        v = ps[:, 0:448].rearrange(
                            "p (yp yb xp two) -> p yp xp yb two", yp=4, yb=2, two=2)
                        m2 = sbim.tile([128, 4, 28], F32, tag="m23")
                        nc.vector.tensor_reduce(m2[:], v, mybir.AxisListType.XY,
                                                MAX)
                        nc.scalar.activation(act3[:, mt, 4 * g:4 * g + 4, :],
                                             m2[:], RELU, bias=b3t[:, mt:mt + 1])
                return act3

            def hout(i, act3):
                # h[k] with k = c*784+s, c = mt*128 + a*32 + b -> shard sh=mt*4+a
                ccv = cc_in.rearrange("(mt a) i (b s) -> mt i a b s", a=4, b=32)
                a3f = act3[:].bitcast(F32)
                for mt in range(2):
                    for a in range(4):
                        nc.sync.dma_start(
                            ccv[mt, i, a],
                            a3f[32 * a:32 * (a + 1), mt]
                            .rearrange("b s1 s2 -> b (s1 s2)"))

            # pipeline: act1/act2 are single-buffered, so emission order
            # matches the intended PE order.
            fp0 = dct(0)
            fp1 = dct(1)
            a1 = conv1(0, fp0)
            prev_fp = fp1
            for i in range(NIMG):
                act2 = conv2(i, a1)
                if i + 2 < NIMG:
                    nfp = dct(i + 2)
                else:
                    nfp = None
                act3 = conv3(i, act2)
                hout(i, act3)
                if i + 1 < NIMG:
                    a1 = conv1(i + 1, prev_fp)
                    prev_fp = nfp

        # ---------------- fc phase ----------------
        with tc.tile_pool(name="fcconst", bufs=1) as fcconst, \
             tc.tile_pool(name="sbfc", bufs=4) as sbfc, \
             tc.tile_pool(name="psfcT", bufs=3, space="PSUM") as psfcT, \
             tc.tile_pool(name="psfcO", bufs=1, space="PSUM") as psfcO:
            if sim_single:
                nc.sync.dma_start(cc_out, cc_in)
            else:
                nc.gpsimd.collective_compute(
                    "AllToAll", mybir.AluOpType.bypass,
                    replica_groups=[list(range(NCORES))],
                    ins=[cc_in], outs=[cc_out])

            ident32 = fcconst.tile([32, 32], F32)
            make_identity(nc, ident32[:])
            psO = psfcO.tile([32, 512], F32)
            ccv = cc_out.rearrange("s i (kt k) -> (s i) kt k", k=128)

            pend = None
            for kb in range(KT // 4):
                hsb = sbfc.tile([32, 4, 128], F32, tag="hsb")
                nc.sync.dma_start(
                    hsb[:], ccv[:, 4 * kb:4 * kb + 4, :])
                wt = sbfc.tile([128, 4, 512], BF16, tag="wt")
                nc.gpsimd.dma_start(
                    wt[:], fcwT[512 * kb:512 * (kb + 1), :]
                    .rearrange("(f p) o -> p f o", p=128))
                for f in range(4):
                    kt = 4 * kb + f
                    pT = psfcT.tile([128, 32], F32, tag="fcT")
                    nc.tensor.transpose(pT[:], hsb[:, f, :], ident32[:])
                    lh = sbfc.tile([128, 32], BF16, tag="lh")
                    nc.vector.tensor_copy(lh[:], pT[:])
                    if pend is not None:
                        plh, pwt, pf, pkt = pend
                        nc.tensor.matmul(psO[:], lhsT=plh[:], rhs=pwt[:, pf, :],
                                         start=(pkt == 0), stop=False)
                    pend = (lh, wt, f, kt)
            plh, pwt, pf, pkt = pend
            nc.tensor.matmul(psO[:], lhsT=plh[:], rhs=pwt[:, pf, :],
                             start=False, stop=True)
            outsb = sbfc.tile([32, 512], F32)
            nc.vector.tensor_copy(outsb[:], psO[:])
            nc.sync.dma_start(partial, outsb[:])

    nc.compile()
    return nc


def _prep_inputs(x, w1, b1, w2, b2, w3, b3, fcw):
    import ml_dtypes
    D = _dct_matrix(N)
    DTt = np.zeros((P, 2, 256), np.float32)
    DTt[:, :, 0:N] = D.T.reshape(2, P, N).transpose(1, 0, 2)

    w1sb = np.ascontiguousarray(
        np.asarray(w1, np.float32).transpose(1, 2, 3, 0).reshape(27, 64))
    w2 = np.asarray(w2, np.float32)
    w2pair = np.empty((128, 3, 128), np.float32)
    w2last = np.empty((64, 3, 128), np.float32)
    for dx in range(3):
        w2pair[0:64, dx, :] = w2[:, :, 0, dx].T
        w2pair[64:128, dx, :] = w2[:, :, 1, dx].T
        w2last[:, dx, :] = w2[:, :, 2, dx].T
    w3sb = np.ascontiguousarray(
        np.asarray(w3, np.float32).transpose(1, 2, 3, 0).reshape(128, 9, 256))
    b3sb = np.ascontiguousarray(np.asarray(b3, np.float32).reshape(2, 128).T)

    x = np.ascontiguousarray(np.asarray(x, np.float32))
    fcw8 = np.asarray(fcw, np.float32).reshape(512, NCORES, KCH)

    in_maps = []
    for j in range(NCORES):
        fcwT_j = np.ascontiguousarray(
            fcw8[:, j, :].T).astype(ml_dtypes.bfloat16)
        in_maps.append({
            "x4": x[4 * j:4 * j + 4],
            "dctT": DTt,
            "w1d": w1sb,
            "w2pd": w2pair,
            "w2ld": w2last,
            "w3d": w3sb,
            "b1d": np.ascontiguousarray(np.asarray(b1, np.float32)[:, None]),
            "b2d": np.ascontiguousarray(np.asarray(b2, np.float32)[:, None]),
            "b3d": b3sb,
            "fcwT": fcwT_j,
        })
    return in_maps


def kernel(x, w1, b1, w2, b2, w3, b3, fcw, fcb, _trace=False):
    from concourse import bass_utils

    if "nc" not in _CACHE:
        _CACHE["nc"] = _build()
    nc = _CACHE["nc"]

    in_maps = _prep_inputs(x, w1, b1, w2, b2, w3, b3, fcw)
    res = bass_utils.run_bass_kernel_spmd(
        nc, in_maps, core_ids=list(range(NCORES)), trace=_trace)
    out = np.zeros((32, 512), np.float32)
    for j in range(NCORES):
        out += res.results[j]["partial"]
    out += np.asarray(fcb, np.float32)[None, :]
    if _trace:
        return out, res
    return out

